# revision 1
# baseline (speedup 1.0000x reference)
"""Trainium2 Bass kernel for ContourProposalNetwork.

Strategy: 8-way spatial sharding (rows) of both conv heads; classification
head in fp32 (exact threshold decisions for the binary map), refinement head
in fp16. Contour refinement + binary-map upsample run as a second small SPMD
kernel using a dense "hat-function" formulation of bilinear sampling
(sample = Ey^T F Ex with Ex[c,p] = relu(1-|x_p-c|)), so no gathers are needed.

kernel(**inputs) takes the full unsharded inputs and returns
(classification, binary_map, contours) exactly like the reference.
"""
import sys
sys.path.insert(0, "/opt/trn_rl_repo")

import numpy as np
import concourse.mybir as mybir
import concourse.tile as tile
from concourse import bacc
from concourse.bass_utils import run_bass_kernel_spmd

F16 = mybir.dt.float16
F32 = mybir.dt.float32
AF = mybir.ActivationFunctionType
ALU = mybir.AluOpType

H1, H2 = 256, 128
W1, W2 = 256, 128
R1, R2 = H1 // 8, H2 // 8
NPTS = 5120  # padded points per core (600/8 instances * 64 pts = 4800)
NREAL = 4800

_cache = {}
last_exec_ns = {"phase1": None, "phase2": None}


def _groups(rows):
    out = []
    r = 0
    while r < rows:
        g = min(4, rows - r)
        out.append((r, g))
        r += g
    return out


# --------------------------------------------------------------------------
# phase 1 builder: conv heads
# --------------------------------------------------------------------------

def build_phase1():
    nc = bacc.Bacc("TRN2", target_bir_lowering=False, debug=False, num_devices=8)

    x2p = nc.dram_tensor("x2p", [512, R2 + 6, W2 + 2], F32, kind="ExternalInput")
    x1p = nc.dram_tensor("x1p", [512, R1 + 6, W1 + 2], F16, kind="ExternalInput")
    wdefs = {
        "wc1": ([128, 9, 4, 1024], F32), "wc2": ([128, 9, 8, 512], F32),
        "wc3": ([128, 9, 4, 256], F32), "wc4": ([128, 2, 1], F32),
        "wr1": ([128, 9, 4, 1024], F16), "wr2": ([128, 9, 8, 512], F16),
        "wr3": ([128, 9, 4, 256], F16), "wr4": ([128, 2, 2], F16),
    }
    wd = {k: nc.dram_tensor(k, sh, dt, kind="ExternalInput")
          for k, (sh, dt) in wdefs.items()}
    bdefs = {"bc1": [128, 8], "bc2": [128, 4], "bc3": [128, 2], "bc4": [1, 1],
             "br1": [128, 8], "br2": [128, 4], "br3": [128, 2], "br4": [2, 1]}
    bd = {k: nc.dram_tensor(k, sh, F32, kind="ExternalInput")
          for k, sh in bdefs.items()}

    class_out = nc.dram_tensor("class_out", [R2, W2], F32, kind="ExternalOutput")
    field_out = nc.dram_tensor("field_out", [2, R1, W1], F32, kind="ExternalOutput")

    h1c = nc.dram_tensor("h1c", [1024, R2 + 4, W2 + 2], F32)
    h2c = nc.dram_tensor("h2c", [512, R2 + 2, W2 + 2], F32)
    h3c = nc.dram_tensor("h3c", [256, R2, W2], F32)
    h1r = nc.dram_tensor("h1r", [1024, R1 + 4, W1 + 2], F16)
    h2r = nc.dram_tensor("h2r", [512, R1 + 2, W1 + 2], F16)
    h3r = nc.dram_tensor("h3r", [256, R1, W1], F16)

    with tile.TileContext(nc) as tc:
        with (
            tc.tile_pool(name="wp", bufs=4) as wp,
            tc.tile_pool(name="win", bufs=3) as win,
            tc.tile_pool(name="stage", bufs=3) as stage,
            tc.tile_pool(name="stage4", bufs=2) as stage4,
            tc.tile_pool(name="cst", bufs=1) as cst,
            tc.tile_pool(name="ps", bufs=6, space="PSUM") as ps,
            tc.tile_pool(name="ps4", bufs=2, space="PSUM") as ps4,
        ):
            btiles = {}
            for k, sh in bdefs.items():
                bt = cst.tile(sh, F32, tag=k)
                nc.sync.dma_start(bt[:], bd[k][:])
                btiles[k] = bt
            w4c = cst.tile([128, 2, 1], F32, tag="w4c")
            nc.sync.dma_start(w4c[:], wd["wc4"][:])
            w4r = cst.tile([128, 2, 2], F16, tag="w4r")
            nc.sync.dma_start(w4r[:], wd["wr4"][:])
            zz = cst.tile([128, R1 + 4, 1], F32, tag="zz")
            nc.vector.memset(zz[:], 0)
            zz16 = cst.tile([128, R1 + 4, 1], F16, tag="zz16")
            nc.vector.memset(zz16[:], 0)
            zr32 = cst.tile([128, 2, W2 + 2], F32, tag="zr32")
            nc.vector.memset(zr32[:], 0)
            zr16 = cst.tile([128, 2, W1 + 2], F16, tag="zr16")
            nc.vector.memset(zr16[:], 0)
            pid = nc.partition_id()

            def zero_rows(dst, nch, rows, nz, dtype):
                src = zr32 if dtype == F32 else zr16
                w = dst.shape[2]
                with tc.If(pid == 0):
                    for cb in range(nch // 128):
                        nc.sync.dma_start(
                            dst[cb * 128:(cb + 1) * 128, 0:nz, :],
                            src[:, :nz, :w])
                with tc.If(pid == 7):
                    for cb in range(nch // 128):
                        nc.sync.dma_start(
                            dst[cb * 128:(cb + 1) * 128, rows - nz:rows, :],
                            src[:, :nz, :w])

            def zero_cols(dst, nch, rows, padw, dtype):
                src = zz if dtype == F32 else zz16
                for cb in range(nch // 128):
                    for side in (0, padw - 1):
                        nc.sync.dma_start(
                            dst[cb * 128:(cb + 1) * 128, :, side:side + 1],
                            src[:, :rows, :])

            def conv3x3(src, dst, wtiles, wslice, bias, Ci, Co, rows_out, W,
                        dt_in, dt_out, act):
                cib_n = Ci // 128
                pad_dst = dst.shape[2] == W + 2
                half = cib_n > 4
                for (g0, gr) in _groups(rows_out):
                    xw = []
                    nload = 2 if half else 1
                    per = cib_n // nload
                    for h in range(nload):
                        t_ = win.tile([128, per, 6, W + 2], dt_in, tag="win")
                        nc.sync.dma_start(
                            t_[:, :, :gr + 2, :],
                            src[h * per * 128:(h + 1) * per * 128,
                                g0:g0 + gr + 2, :].rearrange(
                                    "(b p) r c -> p b r c", p=128))
                        xw.append(t_)
                    for co in range(Co // 128):
                        psums = []
                        for sub in range(0, gr, 2):
                            pt = ps.tile([128, 2, W], F32, tag="ps")
                            psums.append(pt)
                        first = True
                        for t in range(9):
                            dy, dx = t // 3, t % 3
                            for cb in range(cib_n):
                                lhsT = wslice(wtiles, t, cb, co)
                                xt = xw[cb // per][:, cb % per]
                                for si, pt in enumerate(psums):
                                    last = (t == 8 and cb == cib_n - 1)
                                    nc.tensor.matmul(
                                        pt[:], lhsT,
                                        xt[:, dy + 2 * si:dy + 2 * si + 2,
                                           dx:dx + W],
                                        start=first, stop=last)
                                first = False
                        for si, pt in enumerate(psums):
                            st = stage.tile([128, 2, W], dt_out, tag="st")
                            nc.scalar.activation(st[:], pt[:], act,
                                                 bias=bias[:, co:co + 1],
                                                 scale=1.0)
                            r0 = g0 + 2 * si
                            if pad_dst:
                                nc.sync.dma_start(
                                    dst[co * 128:(co + 1) * 128,
                                        r0:r0 + 2, 1:W + 1], st[:])
                            else:
                                nc.sync.dma_start(
                                    dst[co * 128:(co + 1) * 128,
                                        r0:r0 + 2, :], st[:])

            # ---- classification head (fp32) ----
            wc1_t = []
            for cb in range(4):
                wt_c = wp.tile([128, 9, 1024], F32, tag="w")
                nc.sync.dma_start(wt_c[:], wd["wc1"][:, :, cb, :])
                wc1_t.append(wt_c)
            conv3x3(x2p, h1c, wc1_t,
                    lambda ws, t, cb, co: ws[cb][:, t, co * 128:co * 128 + 128],
                    btiles["bc1"], 512, 1024, R2 + 4, W2, F32, F32, AF.Relu)
            zero_cols(h1c, 1024, R2 + 4, W2 + 2, F32)
            zero_rows(h1c, 1024, R2 + 4, 2, F32)

            wc2_t = []
            for pair in range(4):
                wt_c2 = wp.tile([128, 9, 2, 512], F32, tag="w")
                nc.sync.dma_start(wt_c2[:], wd["wc2"][:, :, 2 * pair:2 * pair + 2, :])
                wc2_t.append(wt_c2)
            conv3x3(h1c, h2c, wc2_t,
                    lambda ws, t, cb, co: ws[cb // 2][:, t, cb % 2,
                                                      co * 128:co * 128 + 128],
                    btiles["bc2"], 1024, 512, R2 + 2, W2, F32, F32, AF.Relu)
            zero_cols(h2c, 512, R2 + 2, W2 + 2, F32)
            zero_rows(h2c, 512, R2 + 2, 1, F32)

            wc3_t = []
            for pair in range(2):
                wt_c3 = wp.tile([128, 9, 2, 256], F32, tag="w")
                nc.sync.dma_start(wt_c3[:], wd["wc3"][:, :, 2 * pair:2 * pair + 2, :])
                wc3_t.append(wt_c3)
            conv3x3(h2c, h3c, wc3_t,
                    lambda ws, t, cb, co: ws[cb // 2][:, t, cb % 2,
                                                      co * 128:co * 128 + 128],
                    btiles["bc3"], 512, 256, R2, W2, F32, F32, AF.Relu)

            for (g0, gr) in _groups(R2):
                xw4 = win.tile([128, 2, 4, W2], F32, tag="win")
                nc.sync.dma_start(
                    xw4[:, :, :gr, :],
                    h3c[:, g0:g0 + gr, :].rearrange("(b p) r c -> p b r c", p=128))
                pt4 = ps4.tile([1, 4 * W2], F32, tag="ps4")
                for cb in range(2):
                    nc.tensor.matmul(pt4[:, :gr * W2],
                                     w4c[:, cb, :], xw4[:, cb, :gr, :],
                                     start=(cb == 0), stop=(cb == 1))
                st4 = stage4.tile([1, 4 * W2], F32, tag="st4")
                nc.scalar.activation(st4[:, :gr * W2], pt4[:, :gr * W2],
                                     AF.Sigmoid, bias=btiles["bc4"][:, 0:1],
                                     scale=1.0)
                nc.sync.dma_start(class_out[g0:g0 + gr, :], st4[:, :gr * W2])

            # ---- refinement head (fp16) ----
            wr1_t = []
            for pair in range(2):
                wt_r1 = wp.tile([128, 9, 2, 1024], F16, tag="w")
                nc.sync.dma_start(wt_r1[:], wd["wr1"][:, :, 2 * pair:2 * pair + 2, :])
                wr1_t.append(wt_r1)
            conv3x3(x1p, h1r, wr1_t,
                    lambda ws, t, cb, co: ws[cb // 2][:, t, cb % 2,
                                                      co * 128:co * 128 + 128],
                    btiles["br1"], 512, 1024, R1 + 4, W1, F16, F16, AF.Relu)
            zero_cols(h1r, 1024, R1 + 4, W1 + 2, F16)
            zero_rows(h1r, 1024, R1 + 4, 2, F16)

            wr2_t = []
            for halfi in range(2):
                wt_r2 = wp.tile([128, 9, 4, 512], F16, tag="w")
                nc.sync.dma_start(wt_r2[:], wd["wr2"][:, :, 4 * halfi:4 * halfi + 4, :])
                wr2_t.append(wt_r2)
            conv3x3(h1r, h2r, wr2_t,
                    lambda ws, t, cb, co: ws[cb // 4][:, t, cb % 4,
                                                      co * 128:co * 128 + 128],
                    btiles["br2"], 1024, 512, R1 + 2, W1, F16, F16, AF.Relu)
            zero_cols(h2r, 512, R1 + 2, W1 + 2, F16)
            zero_rows(h2r, 512, R1 + 2, 1, F16)

            wr3_t = [None]
            wt_r3 = wp.tile([128, 9, 4, 256], F16, tag="w")
            nc.sync.dma_start(wt_r3[:], wd["wr3"][:])
            wr3_t[0] = wt_r3
            conv3x3(h2r, h3r, wr3_t,
                    lambda ws, t, cb, co: ws[0][:, t, cb,
                                                co * 128:co * 128 + 128],
                    btiles["br3"], 512, 256, R1, W1, F16, F16, AF.Relu)

            for (g0, gr) in _groups(R1):
                xw4r = win.tile([128, 2, 4, W1], F16, tag="win")
                nc.sync.dma_start(
                    xw4r[:, :, :gr, :],
                    h3r[:, g0:g0 + gr, :].rearrange("(b p) r c -> p b r c", p=128))
                for sub in range(0, gr, 2):
                    pt4r = ps4.tile([2, 2, W1], F32, tag="ps4")
                    for cb in range(2):
                        nc.tensor.matmul(
                            pt4r[:], w4r[:, cb, :],
                            xw4r[:, cb, sub:sub + 2, :],
                            start=(cb == 0), stop=(cb == 1))
                    st4r = stage4.tile([2, 2, W1], F32, tag="st4")
                    nc.scalar.activation(st4r[:], pt4r[:], AF.Identity,
                                         bias=btiles["br4"][:, 0:1], scale=1.0)
                    nc.sync.dma_start(field_out[:, g0 + sub:g0 + sub + 2, :],
                                      st4r[:])
    nc.compile()
    return nc


# --------------------------------------------------------------------------
# phase 2 builder: binary map + contour refinement
# --------------------------------------------------------------------------

def build_phase2(n_iter):
    PGN = NPTS // 512
    nc = bacc.Bacc("TRN2", target_bir_lowering=False, debug=False, num_devices=8)
    classT = nc.dram_tensor("classT", [128, 128], F32, kind="ExternalInput")
    fieldT = nc.dram_tensor("fieldT", [2, 256, 256], F32, kind="ExternalInput")
    cont0 = nc.dram_tensor("cont0", [2, NPTS], F32, kind="ExternalInput")
    uxT = nc.dram_tensor("uxT", [128, 256], F32, kind="ExternalInput")
    uyT = nc.dram_tensor("uyT", [128, 32], F32, kind="ExternalInput")
    iota_neg = nc.dram_tensor("iota_neg", [128, 2], F32, kind="ExternalInput")
    brows = nc.dram_tensor("brows", [32, 256], F32, kind="ExternalOutput")
    cont_out = nc.dram_tensor("cont_out", [2, NPTS], F32, kind="ExternalOutput")
    cont_d = nc.dram_tensor("cont_d", [2, NPTS], F32)

    with tile.TileContext(nc) as tc:
        with (
            tc.tile_pool(name="cst", bufs=1) as cst,
            tc.tile_pool(name="big", bufs=2) as big,
            tc.tile_pool(name="sm", bufs=4) as sm,
            tc.tile_pool(name="ps", bufs=4, space="PSUM") as ps,
            tc.tile_pool(name="ps1", bufs=2, space="PSUM") as ps1,
        ):
            ct = cst.tile([128, 128], F32, tag="classT")
            nc.sync.dma_start(ct[:], classT[:])
            ux = cst.tile([128, 256], F32, tag="uxT")
            nc.sync.dma_start(ux[:], uxT[:])
            uy = cst.tile([128, 32], F32, tag="uyT")
            nc.sync.dma_start(uy[:], uyT[:])
            bt = cst.tile([128, 128], F32, tag="bt")
            nc.vector.tensor_scalar(bt[:], ct[:], 0.5, None, ALU.is_ge)
            o1p = ps.tile([128, 256], F32, tag="ps")
            nc.tensor.matmul(o1p[:], bt[:], ux[:], start=True, stop=True)
            o1 = cst.tile([128, 256], F32, tag="o1")
            nc.scalar.copy(o1[:], o1p[:])
            o2p = ps.tile([32, 256], F32, tag="ps")
            nc.tensor.matmul(o2p[:], uy[:], o1[:], start=True, stop=True)
            br = cst.tile([32, 256], F32, tag="br")
            nc.vector.tensor_scalar(br[:], o2p[:], 0.5, None, ALU.is_ge)
            nc.sync.dma_start(brows[:], br[:])

            ft = cst.tile([128, 2, 2, 256], F32, tag="ft")
            nc.sync.dma_start(
                ft[:], fieldT.rearrange("h (b p) r -> p h b r", p=128))
            inx = cst.tile([128, 2], F32, tag="iota")
            nc.sync.dma_start(inx[:], iota_neg[:])
            ones = cst.tile([128, 1], F32, tag="ones")
            nc.vector.memset(ones[:], 1.0)

            cont = big.tile([33, NPTS], F32, tag="cont")
            nc.sync.dma_start(cont[0:1, :], cont0[0:1, :])
            nc.sync.dma_start(cont[32:33, :], cont0[1:2, :])

            for it in range(n_iter):
                src = cont0 if it == 0 else cont_d
                offs = big.tile([33, NPTS], F32, tag="offs")
                for pg in range(PGN):
                    sl = slice(pg * 512, pg * 512 + 512)
                    ex = sm.tile([128, 2, 512], F32, tag="ex")
                    ey = sm.tile([128, 2, 512], F32, tag="ey")
                    xr = sm.tile([128, 512], F32, tag="xr")
                    nc.sync.dma_start(xr[:], src[0, sl].partition_broadcast(128))
                    yr = sm.tile([128, 512], F32, tag="yr")
                    nc.sync.dma_start(yr[:], src[1, sl].partition_broadcast(128))
                    for cb in range(2):
                        for rep, e in ((xr, ex), (yr, ey)):
                            d = sm.tile([128, 512], F32, tag="d")
                            nc.scalar.activation(d[:], rep[:], AF.Abs,
                                                 bias=inx[:, cb:cb + 1], scale=1.0)
                            nc.scalar.activation(e[:, cb, :], d[:], AF.Relu,
                                                 bias=1.0, scale=-1.0)
                    for ch in range(2):
                        t1a = ps.tile([128, 512], F32, tag="ps")
                        t1b = ps.tile([128, 512], F32, tag="ps")
                        t1 = [t1a, t1b]
                        for rb in range(2):
                            for cb in range(2):
                                nc.tensor.matmul(
                                    t1[rb][:],
                                    ft[:, ch, cb, rb * 128:rb * 128 + 128],
                                    ex[:, cb, :],
                                    start=(cb == 0), stop=(cb == 1))
                        op = ps1.tile([1, 512], F32, tag="ps1")
                        for rb in range(2):
                            m = sm.tile([128, 512], F32, tag="m")
                            nc.vector.tensor_mul(m[:], t1[rb][:], ey[:, rb, :])
                            nc.tensor.matmul(op[:], ones[:], m[:],
                                             start=(rb == 0), stop=(rb == 1))
                        nc.scalar.copy(offs[32 * ch:32 * ch + 1, sl], op[:])
                cnew = big.tile([33, NPTS], F32, tag="cont")
                for ch in range(2):
                    r = slice(32 * ch, 32 * ch + 1)
                    nc.vector.tensor_add(cnew[r, :], cont[r, :], offs[r, :])
                    nc.vector.tensor_scalar(cnew[r, :], cnew[r, :], 0.0, 255.0,
                                            ALU.max, ALU.min)
                cont = cnew
                dst = cont_d if it < n_iter - 1 else cont_out
                nc.sync.dma_start(dst[0:1, :], cont[0:1, :])
                nc.sync.dma_start(dst[1:2, :], cont[32:33, :])
    nc.compile()
    return nc


# --------------------------------------------------------------------------
# host-side helpers
# --------------------------------------------------------------------------

def _wlayout3(w):
    Co, Ci = w.shape[0], w.shape[1]
    a = np.transpose(w, (2, 3, 1, 0)).reshape(9, Ci // 128, 128, Co)
    return np.ascontiguousarray(np.transpose(a, (2, 0, 1, 3)))


def _wlayout1(w):
    Co, Ci = w.shape[0], w.shape[1]
    a = w.reshape(Co, Ci // 128, 128)
    return np.ascontiguousarray(np.transpose(a, (2, 1, 0)))


def _blayout(b):
    n = b.shape[0]
    if n < 128:
        return np.ascontiguousarray(b.reshape(n, 1).astype(np.float32))
    return np.ascontiguousarray(b.reshape(n // 128, 128).T.astype(np.float32))


def _shard_features(f, R, halo=3):
    C, H, W = f.shape
    out = []
    for i in range(8):
        sl = np.zeros((C, R + 2 * halo, W + 2), np.float32)
        lo, hi = R * i - halo, R * i + R + halo
        clo, chi = max(lo, 0), min(hi, H)
        sl[:, clo - lo:chi - lo, 1:W + 1] = f[:, clo:chi, :]
        out.append(sl)
    return out


def _upsample_weights():
    """Ux^T [128, 256] replicating the reference's fp32 weight values."""
    xs = np.linspace(0.0, 127.0, 256).astype(np.float32)
    x0 = np.floor(xs).astype(np.int32)
    x1 = np.minimum(x0 + 1, 127)
    wx = (xs - x0).astype(np.float32)
    uxT = np.zeros((128, 256), np.float32)
    for i in range(256):
        uxT[x0[i], i] += (np.float32(1.0) - wx[i])
        uxT[x1[i], i] += wx[i]
    return uxT


def kernel(**inputs):
    f1 = np.asarray(inputs["features_1"], np.float32)[0]   # (512,256,256)
    f2 = np.asarray(inputs["features_2"], np.float32)[0]   # (512,128,128)
    cont = np.asarray(inputs["coarse_contours"], np.float32)[0]  # (600,64,2)
    n_iter = int(np.asarray(inputs["num_iter"]))

    if "p1" not in _cache:
        _cache["p1"] = build_phase1()
    if ("p2", n_iter) not in _cache:
        _cache[("p2", n_iter)] = build_phase2(n_iter)
    nc1 = _cache["p1"]
    nc2 = _cache[("p2", n_iter)]

    wglob = {
        "wc1": _wlayout3(np.asarray(inputs["cw1"], np.float32)),
        "wc2": _wlayout3(np.asarray(inputs["cw2"], np.float32)),
        "wc3": _wlayout3(np.asarray(inputs["cw3"], np.float32)),
        "wc4": _wlayout1(np.asarray(inputs["cw4"], np.float32)),
        "wr1": _wlayout3(np.asarray(inputs["rw1"], np.float32)).astype(np.float16),
        "wr2": _wlayout3(np.asarray(inputs["rw2"], np.float32)).astype(np.float16),
        "wr3": _wlayout3(np.asarray(inputs["rw3"], np.float32)).astype(np.float16),
        "wr4": _wlayout1(np.asarray(inputs["rw4"], np.float32)).astype(np.float16),
        "bc1": _blayout(np.asarray(inputs["cb1"])), "bc2": _blayout(np.asarray(inputs["cb2"])),
        "bc3": _blayout(np.asarray(inputs["cb3"])), "bc4": _blayout(np.asarray(inputs["cb4"])),
        "br1": _blayout(np.asarray(inputs["rb1"])), "br2": _blayout(np.asarray(inputs["rb2"])),
        "br3": _blayout(np.asarray(inputs["rb3"])), "br4": _blayout(np.asarray(inputs["rb4"])),
    }
    x2s = _shard_features(f2, R2)
    x1s = _shard_features(f1, R1)
    in1 = []
    for i in range(8):
        m = dict(wglob)
        m["x2p"] = x2s[i]
        m["x1p"] = x1s[i].astype(np.float16)
        in1.append(m)

    res1 = run_bass_kernel_spmd(nc1, in1, core_ids=list(range(8)))
    last_exec_ns["phase1"] = res1.exec_time_ns
    classification = np.concatenate(
        [res1.results[i]["class_out"] for i in range(8)], 0)      # (128,128)
    field = np.concatenate(
        [res1.results[i]["field_out"] for i in range(8)], 1)      # (2,256,256)

    uxT = _upsample_weights()
    iota = -(np.arange(128, dtype=np.float32)[:, None]
             + np.array([0.0, 128.0], np.float32)[None, :])
    classT = np.ascontiguousarray(classification.T)
    fieldT = np.ascontiguousarray(np.transpose(field, (0, 2, 1)))
    in2 = []
    for i in range(8):
        pts = cont[75 * i:75 * (i + 1)].reshape(NREAL, 2)
        c0 = np.zeros((2, NPTS), np.float32)
        c0[0, :NREAL] = pts[:, 0]
        c0[1, :NREAL] = pts[:, 1]
        in2.append({
            "classT": classT, "fieldT": fieldT, "cont0": c0, "uxT": uxT,
            "uyT": np.ascontiguousarray(uxT[:, 32 * i:32 * (i + 1)]),
            "iota_neg": np.ascontiguousarray(iota.astype(np.float32)),
        })

    res2 = run_bass_kernel_spmd(nc2, in2, core_ids=list(range(8)))
    last_exec_ns["phase2"] = res2.exec_time_ns
    binary_map = np.concatenate(
        [res2.results[i]["brows"] for i in range(8)], 0)          # (256,256)
    conts = []
    for i in range(8):
        co = res2.results[i]["cont_out"]
        conts.append(np.stack([co[0, :NREAL], co[1, :NREAL]], -1).reshape(75, 64, 2))
    contours = np.concatenate(conts, 0)                           # (600,64,2)

    return (classification.reshape(1, 1, 128, 128).astype(np.float32),
            binary_map.reshape(1, 1, 256, 256).astype(np.float32),
            contours.reshape(1, 600, 64, 2).astype(np.float32))


# revision 6
# speedup vs baseline: 1.2921x; 1.2921x over previous
"""Trainium2 Bass kernel for ContourProposalNetwork.

Strategy: 8-way spatial sharding (rows) of both conv heads; classification
head in fp32 (exact threshold decisions for the binary map), refinement head
in fp16. Contour refinement + binary-map upsample run as a second small SPMD
kernel using a dense "hat-function" formulation of bilinear sampling
(sample = Ey^T F Ex with Ex[c,p] = relu(1-|x_p-c|)), so no gathers are needed.

kernel(**inputs) takes the full unsharded inputs and returns
(classification, binary_map, contours) exactly like the reference.
"""
import sys
sys.path.insert(0, "/opt/trn_rl_repo")

import numpy as np

# Register the antenv.axon_hooks stub if missing so run_bass_kernel_spmd with
# BASS_TRACE set cannot crash on import (the agent image lacks this module).
try:
    import antenv.axon_hooks  # noqa: F401
except Exception:
    try:
        import types as _types
        import antenv as _antenv
        _m = _types.ModuleType("antenv.axon_hooks")
        _m._hook = None
        _m.set_axon_ntff_profile_hook = lambda h: setattr(_m, "_hook", h)
        _m.get_axon_ntff_profile_hook = lambda: _m._hook
        sys.modules["antenv.axon_hooks"] = _m
        _antenv.axon_hooks = _m
        try:
            from trn_agent_boot.trn_boot import _ntff_profile_via_ctypes
            _h = _ntff_profile_via_ctypes("/opt/axon/libaxon_pjrt.so")
            if _h is not None:
                _m.set_axon_ntff_profile_hook(_h)
        except Exception:
            pass
    except Exception:
        pass

import concourse.mybir as mybir
import concourse.tile as tile
from concourse import bacc
from concourse.bass_utils import run_bass_kernel_spmd

F16 = mybir.dt.float16
F32 = mybir.dt.float32
AF = mybir.ActivationFunctionType
ALU = mybir.AluOpType

H1, H2 = 256, 128
W1, W2 = 256, 128
R1, R2 = H1 // 8, H2 // 8
NPTS = 5120  # padded points per core (600/8 instances * 64 pts = 4800)
NREAL = 4800

_cache = {}
last_exec_ns = {"phase1": None, "phase2": None}


def _groups(rows):
    out = []
    r = 0
    while r < rows:
        g = min(4, rows - r)
        out.append((r, g))
        r += g
    return out


# --------------------------------------------------------------------------
# phase 1 builder: conv heads
# --------------------------------------------------------------------------

def build_phase1(h1=H1, h2=H2):
    W1_, W2_ = 256, 128
    r1, r2 = h1 // 8, h2 // 8
    nc = bacc.Bacc("TRN2", target_bir_lowering=False, debug=False, num_devices=8)

    x2ph = nc.dram_tensor("x2ph", [512, r2 + 6, W2_ + 2], F16, kind="ExternalInput")
    x2pl = nc.dram_tensor("x2pl", [512, r2 + 6, W2_ + 2], F16, kind="ExternalInput")
    x1p = nc.dram_tensor("x1p", [512, r1 + 6, W1_ + 2], F16, kind="ExternalInput")
    # class weights: fp16 hi/lo chunks; flat layout per chunk:
    #   idx = ((hl*9 + t)*cl_n + cl)*Co + co   (cl = cib within chunk)
    wdefs = {
        "wc1": ([128, 4, 2 * 9 * 1 * 1024], F16),
        "wc2": ([128, 4, 2 * 9 * 2 * 512], F16),
        "wc3": ([128, 2, 2 * 9 * 2 * 256], F16),
        "wc4": ([128, 4], F16),
        "wr1": ([128, 9, 4, 1024], F16), "wr2": ([128, 9, 8, 512], F16),
        "wr3": ([128, 9, 4, 256], F16), "wr4": ([128, 2, 2], F16),
    }
    wd = {k: nc.dram_tensor(k, sh, dt, kind="ExternalInput")
          for k, (sh, dt) in wdefs.items()}
    bdefs = {"bc1": [128, 8], "bc2": [128, 4], "bc3": [128, 2], "bc4": [1, 1],
             "br1": [128, 8], "br2": [128, 4], "br3": [128, 2], "br4": [2, 1]}
    bd = {k: nc.dram_tensor(k, sh, F32, kind="ExternalInput")
          for k, sh in bdefs.items()}

    class_out = nc.dram_tensor("class_out", [r2, W2_], F32, kind="ExternalOutput")
    field_out = nc.dram_tensor("field_out", [2, r1, W1_], F32, kind="ExternalOutput")

    h1c = nc.dram_tensor("h1c", [2, 1024, r2 + 4, W2_ + 2], F16)
    h2c = nc.dram_tensor("h2c", [2, 512, r2 + 2, W2_ + 2], F16)
    h3c = nc.dram_tensor("h3c", [2, 256, r2, W2_], F16)
    h1r = nc.dram_tensor("h1r", [1024, r1 + 4, W1_ + 2], F16)
    h2r = nc.dram_tensor("h2r", [512, r1 + 2, W1_ + 2], F16)
    h3r = nc.dram_tensor("h3r", [256, r1, W1_], F16)

    with tile.TileContext(nc) as tc:
        with (
            tc.tile_pool(name="wp", bufs=4) as wp,
            tc.tile_pool(name="win", bufs=3) as win,
            tc.tile_pool(name="stage", bufs=3) as stage,
            tc.tile_pool(name="stage4", bufs=2) as stage4,
            tc.tile_pool(name="cst", bufs=1) as cst,
            tc.tile_pool(name="ps", bufs=6, space="PSUM") as ps,
            tc.tile_pool(name="ps4", bufs=2, space="PSUM") as ps4,
        ):
            btiles = {}
            for k, sh in bdefs.items():
                bt = cst.tile(sh, F32, tag=k)
                nc.sync.dma_start(bt[:], bd[k][:])
                btiles[k] = bt
            w4c = cst.tile([128, 4], F16, tag="w4c")
            nc.sync.dma_start(w4c[:], wd["wc4"][:])
            w4r = cst.tile([128, 2, 2], F16, tag="w4r")
            nc.sync.dma_start(w4r[:], wd["wr4"][:])
            zz16 = cst.tile([128, r1 + 4, 1], F16, tag="zz16")
            nc.vector.memset(zz16[:], 0)
            zr16 = cst.tile([128, 2, W1_ + 2], F16, tag="zr16")
            nc.vector.memset(zr16[:], 0)
            pid = nc.partition_id()

            def zero_rows(dsts, nch, rows, nz):
                w = dsts[0].shape[-1]
                with tc.If(pid == 0):
                    for d in dsts:
                        for cb in range(nch // 128):
                            nc.sync.dma_start(
                                d[cb * 128:(cb + 1) * 128, 0:nz, :],
                                zr16[:, :nz, :w])
                with tc.If(pid == 7):
                    for d in dsts:
                        for cb in range(nch // 128):
                            nc.sync.dma_start(
                                d[cb * 128:(cb + 1) * 128, rows - nz:rows, :],
                                zr16[:, :nz, :w])

            def zero_cols(dsts, nch, rows, padw):
                for d in dsts:
                    for cb in range(nch // 128):
                        for side in (0, padw - 1):
                            nc.sync.dma_start(
                                d[cb * 128:(cb + 1) * 128, :, side:side + 1],
                                zz16[:, :rows, :])

            def conv3x3(srcs, dsts, wtiles, wslice, bias, Ci, Co, rows_out, W,
                        act, split):
                """srcs: list of DRAM [Ci, rows_out+2, W+2] (1 or hi/lo pair).
                dsts: list of DRAM [Co, rows_out, W+2|W] (1 or hi/lo pair).
                split: 3-pass hi/lo matmuls + hi/lo output split."""
                cib_n = Ci // 128
                padw = dsts[0].shape[-1]
                pad_dst = padw == W + 2
                nload = 2 if cib_n * 6 * (W + 2) * 2 > 12480 else 1
                per = cib_n // nload
                rpm = 512 // W  # rows per matmul
                for (g0, gr) in _groups(rows_out):
                    xw = []  # xw[src][load]
                    for si_, sr in enumerate(srcs):
                        row = []
                        for hld in range(nload):
                            t_ = win.tile([128, per, 6, W + 2], F16, tag="win")
                            nc.sync.dma_start(
                                t_[:, :, :gr + 2, :],
                                sr[hld * per * 128:(hld + 1) * per * 128,
                                   g0:g0 + gr + 2, :].rearrange(
                                       "(b p) r c -> p b r c", p=128))
                            row.append(t_)
                        xw.append(row)
                    for co in range(Co // 128):
                        psums = []
                        subs = []
                        for sub in range(0, gr, rpm):
                            rr = min(rpm, gr - sub)
                            pt = ps.tile([128, rpm, W], F32, tag="ps")
                            psums.append(pt)
                            subs.append((sub, rr))
                        first = True
                        for t in range(9):
                            dy, dx = t // 3, t % 3
                            for cb in range(cib_n):
                                if split:
                                    passes = ((0, 0), (0, 1), (1, 0))
                                else:
                                    passes = ((0, 0),)
                                for pi_, (hl, xi) in enumerate(passes):
                                    lhsT = wslice(wtiles, t, cb, co, hl)
                                    xt = xw[xi][cb // per][:, cb % per]
                                    for (pt, (sub, rr)) in zip(psums, subs):
                                        last = (t == 8 and cb == cib_n - 1
                                                and pi_ == len(passes) - 1)
                                        nc.tensor.matmul(
                                            pt[:, :rr, :], lhsT,
                                            xt[:, dy + sub:dy + sub + rr,
                                               dx:dx + W],
                                            start=(first and pi_ == 0),
                                            stop=last)
                                first = False
                        for (pt, (sub, rr)) in zip(psums, subs):
                            for c0 in range(0, rr, 2):
                                cr = min(2, rr - c0)
                                r0 = g0 + sub + c0
                                psl = pt[:, c0:c0 + cr, :]
                                dsl = (slice(co * 128, co * 128 + 128),
                                       slice(r0, r0 + cr),
                                       slice(1, W + 1) if pad_dst else slice(0, W))
                                if split:
                                    st32 = stage.tile([128, 2, W], F32, tag="st32")
                                    nc.scalar.activation(st32[:, :cr, :], psl,
                                                         act, bias=bias[:, co:co + 1],
                                                         scale=1.0)
                                    hi16 = stage.tile([128, 2, W], F16, tag="hi16")
                                    nc.vector.tensor_copy(hi16[:, :cr, :], st32[:, :cr, :])
                                    hi32 = stage.tile([128, 2, W], F32, tag="hi32")
                                    nc.scalar.copy(hi32[:, :cr, :], hi16[:, :cr, :])
                                    lo16 = stage.tile([128, 2, W], F16, tag="lo16")
                                    nc.vector.tensor_sub(lo16[:, :cr, :], st32[:, :cr, :],
                                                         hi32[:, :cr, :])
                                    nc.sync.dma_start(dsts[0][dsl], hi16[:, :cr, :])
                                    nc.sync.dma_start(dsts[1][dsl], lo16[:, :cr, :])
                                else:
                                    st = stage.tile([128, 2, W], F16, tag="st")
                                    nc.scalar.activation(st[:, :cr, :], psl,
                                                         act, bias=bias[:, co:co + 1],
                                                         scale=1.0)
                                    nc.sync.dma_start(dsts[0][dsl], st[:, :cr, :])

            # ---- classification head (fp16 hi/lo 3-pass) ----
            def cw_slice(chunk_cl):
                def f(ws, t, cb, co, hl):
                    tile_ = ws[cb // chunk_cl]
                    cl = cb % chunk_cl
                    Co_ = ws_co[0]
                    off = ((hl * 9 + t) * chunk_cl + cl) * Co_ + co * 128
                    return tile_[:, off:off + 128]
                return f

            wc1_t = []
            for cb in range(4):
                wt_c = wp.tile([128, 2 * 9 * 1024], F16, tag="w")
                nc.sync.dma_start(wt_c[:], wd["wc1"][:, cb, :])
                wc1_t.append(wt_c)
            ws_co = [1024]
            conv3x3([x2ph, x2pl], [h1c[0], h1c[1]], wc1_t, cw_slice(1),
                    btiles["bc1"], 512, 1024, r2 + 4, W2_, AF.Relu, True)
            zero_cols([h1c[0], h1c[1]], 1024, r2 + 4, W2_ + 2)
            zero_rows([h1c[0], h1c[1]], 1024, r2 + 4, 2)

            wc2_t = []
            for pair in range(4):
                wt_c2 = wp.tile([128, 2 * 9 * 2 * 512], F16, tag="w")
                nc.sync.dma_start(wt_c2[:], wd["wc2"][:, pair, :])
                wc2_t.append(wt_c2)
            ws_co = [512]
            conv3x3([h1c[0], h1c[1]], [h2c[0], h2c[1]], wc2_t, cw_slice(2),
                    btiles["bc2"], 1024, 512, r2 + 2, W2_, AF.Relu, True)
            zero_cols([h2c[0], h2c[1]], 512, r2 + 2, W2_ + 2)
            zero_rows([h2c[0], h2c[1]], 512, r2 + 2, 1)

            wc3_t = []
            for pair in range(2):
                wt_c3 = wp.tile([128, 2 * 9 * 2 * 256], F16, tag="w")
                nc.sync.dma_start(wt_c3[:], wd["wc3"][:, pair, :])
                wc3_t.append(wt_c3)
            ws_co = [256]
            conv3x3([h2c[0], h2c[1]], [h3c[0], h3c[1]], wc3_t, cw_slice(2),
                    btiles["bc3"], 512, 256, r2, W2_, AF.Relu, True)

            # L4: 1x1 256 -> 1, 3-pass, sigmoid
            for (g0, gr) in _groups(r2):
                xw4h = win.tile([128, 2, 4, W2_], F16, tag="win")
                nc.sync.dma_start(
                    xw4h[:, :, :gr, :],
                    h3c[0, :, g0:g0 + gr, :].rearrange("(b p) r c -> p b r c", p=128))
                xw4l = win.tile([128, 2, 4, W2_], F16, tag="win")
                nc.sync.dma_start(
                    xw4l[:, :, :gr, :],
                    h3c[1, :, g0:g0 + gr, :].rearrange("(b p) r c -> p b r c", p=128))
                pt4 = ps4.tile([1, 4 * W2_], F32, tag="ps4")
                first4 = True
                for cb in range(2):
                    for (hl, xv) in ((0, xw4h), (0, xw4l), (1, xw4h)):
                        nc.tensor.matmul(pt4[:, :gr * W2_],
                                         w4c[:, hl * 2 + cb:hl * 2 + cb + 1],
                                         xv[:, cb, :gr, :],
                                         start=first4,
                                         stop=(cb == 1 and hl == 1))
                        first4 = False
                st4 = stage4.tile([1, 4 * W2_], F32, tag="st4")
                nc.scalar.activation(st4[:, :gr * W2_], pt4[:, :gr * W2_],
                                     AF.Sigmoid, bias=btiles["bc4"][:, 0:1],
                                     scale=1.0)
                nc.sync.dma_start(class_out[g0:g0 + gr, :], st4[:, :gr * W2_])

            # ---- refinement head (fp16 single pass) ----
            wr1_t = []
            for pair in range(2):
                wt_r1 = wp.tile([128, 9, 2, 1024], F16, tag="w")
                nc.sync.dma_start(wt_r1[:], wd["wr1"][:, :, 2 * pair:2 * pair + 2, :])
                wr1_t.append(wt_r1)
            conv3x3([x1p], [h1r], wr1_t,
                    lambda ws, t, cb, co, hl: ws[cb // 2][:, t, cb % 2,
                                                          co * 128:co * 128 + 128],
                    btiles["br1"], 512, 1024, r1 + 4, W1_, AF.Relu, False)
            zero_cols([h1r], 1024, r1 + 4, W1_ + 2)
            zero_rows([h1r], 1024, r1 + 4, 2)

            wr2_t = []
            for halfi in range(2):
                wt_r2 = wp.tile([128, 9, 4, 512], F16, tag="w")
                nc.sync.dma_start(wt_r2[:], wd["wr2"][:, :, 4 * halfi:4 * halfi + 4, :])
                wr2_t.append(wt_r2)
            conv3x3([h1r], [h2r], wr2_t,
                    lambda ws, t, cb, co, hl: ws[cb // 4][:, t, cb % 4,
                                                          co * 128:co * 128 + 128],
                    btiles["br2"], 1024, 512, r1 + 2, W1_, AF.Relu, False)
            zero_cols([h2r], 512, r1 + 2, W1_ + 2)
            zero_rows([h2r], 512, r1 + 2, 1)

            wr3_t = [None]
            wt_r3 = wp.tile([128, 9, 4, 256], F16, tag="w")
            nc.sync.dma_start(wt_r3[:], wd["wr3"][:])
            wr3_t[0] = wt_r3
            conv3x3([h2r], [h3r], wr3_t,
                    lambda ws, t, cb, co, hl: ws[0][:, t, cb,
                                                    co * 128:co * 128 + 128],
                    btiles["br3"], 512, 256, r1, W1_, AF.Relu, False)

            for (g0, gr) in _groups(r1):
                xw4r = win.tile([128, 2, 4, W1_], F16, tag="win")
                nc.sync.dma_start(
                    xw4r[:, :, :gr, :],
                    h3r[:, g0:g0 + gr, :].rearrange("(b p) r c -> p b r c", p=128))
                for sub in range(0, gr, 2):
                    pt4r = ps4.tile([2, 2, W1_], F32, tag="ps4")
                    for cb in range(2):
                        nc.tensor.matmul(
                            pt4r[:], w4r[:, cb, :],
                            xw4r[:, cb, sub:sub + 2, :],
                            start=(cb == 0), stop=(cb == 1))
                    st4r = stage4.tile([2, 2, W1_], F32, tag="st4")
                    nc.scalar.activation(st4r[:], pt4r[:], AF.Identity,
                                         bias=btiles["br4"][:, 0:1], scale=1.0)
                    nc.sync.dma_start(field_out[:, g0 + sub:g0 + sub + 2, :],
                                      st4r[:])
    nc.compile()
    return nc


# --------------------------------------------------------------------------
# phase 2 builder: binary map + contour refinement
# --------------------------------------------------------------------------

def build_phase2(n_iter):
    PGN = NPTS // 512
    nc = bacc.Bacc("TRN2", target_bir_lowering=False, debug=False, num_devices=8)
    classT = nc.dram_tensor("classT", [128, 128], F32, kind="ExternalInput")
    fieldT = nc.dram_tensor("fieldT", [2, 256, 256], F16, kind="ExternalInput")
    cont0 = nc.dram_tensor("cont0", [2, NPTS], F32, kind="ExternalInput")
    uxT = nc.dram_tensor("uxT", [128, 256], F32, kind="ExternalInput")
    uyT = nc.dram_tensor("uyT", [128, 32], F32, kind="ExternalInput")
    iota_neg = nc.dram_tensor("iota_neg", [128, 2], F32, kind="ExternalInput")
    brows = nc.dram_tensor("brows", [32, 256], F32, kind="ExternalOutput")
    cont_out = nc.dram_tensor("cont_out", [2, NPTS], F32, kind="ExternalOutput")
    cont_d = nc.dram_tensor("cont_d", [2, NPTS], F32)

    with tile.TileContext(nc) as tc:
        with (
            tc.tile_pool(name="cst", bufs=1) as cst,
            tc.tile_pool(name="big", bufs=2) as big,
            tc.tile_pool(name="sm", bufs=4) as sm,
            tc.tile_pool(name="ps", bufs=4, space="PSUM") as ps,
            tc.tile_pool(name="ps1", bufs=2, space="PSUM") as ps1,
        ):
            ct = cst.tile([128, 128], F32, tag="classT")
            nc.sync.dma_start(ct[:], classT[:])
            ux = cst.tile([128, 256], F32, tag="uxT")
            nc.sync.dma_start(ux[:], uxT[:])
            uy = cst.tile([128, 32], F32, tag="uyT")
            nc.sync.dma_start(uy[:], uyT[:])
            bt = cst.tile([128, 128], F32, tag="bt")
            nc.vector.tensor_scalar(bt[:], ct[:], 0.5, None, ALU.is_ge)
            o1p = ps.tile([128, 256], F32, tag="ps")
            nc.tensor.matmul(o1p[:], bt[:], ux[:], start=True, stop=True)
            o1 = cst.tile([128, 256], F32, tag="o1")
            nc.scalar.copy(o1[:], o1p[:])
            o2p = ps.tile([32, 256], F32, tag="ps")
            nc.tensor.matmul(o2p[:], uy[:], o1[:], start=True, stop=True)
            br = cst.tile([32, 256], F32, tag="br")
            nc.vector.tensor_scalar(br[:], o2p[:], 0.5, None, ALU.is_ge)
            nc.sync.dma_start(brows[:], br[:])

            ft = cst.tile([128, 2, 2, 256], F16, tag="ft")
            nc.sync.dma_start(
                ft[:], fieldT.rearrange("h (b p) r -> p h b r", p=128))
            inx = cst.tile([128, 2], F32, tag="iota")
            nc.sync.dma_start(inx[:], iota_neg[:])
            ones = cst.tile([128, 1], F16, tag="ones")
            nc.vector.memset(ones[:], 1.0)

            cont = big.tile([33, NPTS], F32, tag="cont")
            nc.sync.dma_start(cont[0:1, :], cont0[0:1, :])
            nc.sync.dma_start(cont[32:33, :], cont0[1:2, :])

            for it in range(n_iter):
                src = cont0 if it == 0 else cont_d
                offs = big.tile([33, NPTS], F32, tag="offs")
                for pg in range(PGN):
                    sl = slice(pg * 512, pg * 512 + 512)
                    ex = sm.tile([128, 2, 512], F16, tag="ex")
                    ey = sm.tile([128, 2, 512], F32, tag="ey")
                    xr = sm.tile([128, 512], F32, tag="xr")
                    nc.sync.dma_start(xr[:], src[0, sl].partition_broadcast(128))
                    yr = sm.tile([128, 512], F32, tag="yr")
                    nc.sync.dma_start(yr[:], src[1, sl].partition_broadcast(128))
                    for cb in range(2):
                        for rep, e in ((xr, ex), (yr, ey)):
                            d = sm.tile([128, 512], F32, tag="d")
                            nc.scalar.activation(d[:], rep[:], AF.Abs,
                                                 bias=inx[:, cb:cb + 1], scale=1.0)
                            nc.scalar.activation(e[:, cb, :], d[:], AF.Relu,
                                                 bias=1.0, scale=-1.0)
                    for ch in range(2):
                        t1a = ps.tile([128, 512], F32, tag="ps")
                        t1b = ps.tile([128, 512], F32, tag="ps")
                        t1 = [t1a, t1b]
                        for rb in range(2):
                            for cb in range(2):
                                nc.tensor.matmul(
                                    t1[rb][:],
                                    ft[:, ch, cb, rb * 128:rb * 128 + 128],
                                    ex[:, cb, :],
                                    start=(cb == 0), stop=(cb == 1))
                        op = ps1.tile([1, 512], F32, tag="ps1")
                        for rb in range(2):
                            m = sm.tile([128, 512], F16, tag="m")
                            nc.vector.tensor_mul(m[:], t1[rb][:], ey[:, rb, :])
                            nc.tensor.matmul(op[:], ones[:], m[:],
                                             start=(rb == 0), stop=(rb == 1))
                        nc.scalar.copy(offs[32 * ch:32 * ch + 1, sl], op[:])
                cnew = big.tile([33, NPTS], F32, tag="cont")
                for ch in range(2):
                    r = slice(32 * ch, 32 * ch + 1)
                    nc.vector.tensor_add(cnew[r, :], cont[r, :], offs[r, :])
                    nc.vector.tensor_scalar(cnew[r, :], cnew[r, :], 0.0, 255.0,
                                            ALU.max, ALU.min)
                cont = cnew
                dst = cont_d if it < n_iter - 1 else cont_out
                nc.sync.dma_start(dst[0:1, :], cont[0:1, :])
                nc.sync.dma_start(dst[1:2, :], cont[32:33, :])
    nc.compile()
    return nc


# --------------------------------------------------------------------------
# host-side helpers
# --------------------------------------------------------------------------

def _wlayout3_split(w, cl_n):
    """fp16 hi/lo chunked class-weight layout [128, nchunk, 2*9*cl_n*Co]."""
    Co, Ci = w.shape[0], w.shape[1]
    whi = w.astype(np.float16)
    wlo = (w - whi.astype(np.float32)).astype(np.float16)
    hl = np.stack([whi, wlo], 0)                      # [2, Co, Ci, 3, 3]
    a = np.transpose(hl, (0, 3, 4, 2, 1))             # [2, ky, kx, Ci, Co]
    nchunk = Ci // 128 // cl_n
    a = a.reshape(2, 9, nchunk, cl_n, 128, Co)
    a = np.transpose(a, (4, 2, 0, 1, 3, 5))           # [p, chunk, hl, t, cl, co]
    return np.ascontiguousarray(a.reshape(128, nchunk, 2 * 9 * cl_n * Co))


def _wlayout1_split(w):
    whi = w.astype(np.float16)
    wlo = (w - whi.astype(np.float32)).astype(np.float16)
    out = np.zeros((128, 4), np.float16)
    for hl, ww in enumerate([whi, wlo]):
        for cb in range(2):
            out[:, hl * 2 + cb] = ww[0, cb * 128:(cb + 1) * 128, 0, 0]
    return out


def _wlayout3(w):
    Co, Ci = w.shape[0], w.shape[1]
    a = np.transpose(w, (2, 3, 1, 0)).reshape(9, Ci // 128, 128, Co)
    return np.ascontiguousarray(np.transpose(a, (2, 0, 1, 3)))


def _wlayout1(w):
    Co, Ci = w.shape[0], w.shape[1]
    a = w.reshape(Co, Ci // 128, 128)
    return np.ascontiguousarray(np.transpose(a, (2, 1, 0)))


def _blayout(b):
    n = b.shape[0]
    if n < 128:
        return np.ascontiguousarray(b.reshape(n, 1).astype(np.float32))
    return np.ascontiguousarray(b.reshape(n // 128, 128).T.astype(np.float32))


def _shard_features(f, R, halo=3):
    C, H, W = f.shape
    out = []
    for i in range(8):
        sl = np.zeros((C, R + 2 * halo, W + 2), np.float32)
        lo, hi = R * i - halo, R * i + R + halo
        clo, chi = max(lo, 0), min(hi, H)
        sl[:, clo - lo:chi - lo, 1:W + 1] = f[:, clo:chi, :]
        out.append(sl)
    return out


def _upsample_weights():
    """Ux^T [128, 256] replicating the reference's fp32 weight values."""
    xs = np.linspace(0.0, 127.0, 256).astype(np.float32)
    x0 = np.floor(xs).astype(np.int32)
    x1 = np.minimum(x0 + 1, 127)
    wx = (xs - x0).astype(np.float32)
    uxT = np.zeros((128, 256), np.float32)
    for i in range(256):
        uxT[x0[i], i] += (np.float32(1.0) - wx[i])
        uxT[x1[i], i] += wx[i]
    return uxT


def kernel(**inputs):
    f1 = np.asarray(inputs["features_1"], np.float32)[0]   # (512,256,256)
    f2 = np.asarray(inputs["features_2"], np.float32)[0]   # (512,128,128)
    cont = np.asarray(inputs["coarse_contours"], np.float32)[0]  # (600,64,2)
    n_iter = int(np.asarray(inputs["num_iter"]))

    if "p1" not in _cache:
        _cache["p1"] = build_phase1()
    if ("p2", n_iter) not in _cache:
        _cache[("p2", n_iter)] = build_phase2(n_iter)
    nc1 = _cache["p1"]
    nc2 = _cache[("p2", n_iter)]

    wglob = {
        "wc1": _wlayout3_split(np.asarray(inputs["cw1"], np.float32), 1),
        "wc2": _wlayout3_split(np.asarray(inputs["cw2"], np.float32), 2),
        "wc3": _wlayout3_split(np.asarray(inputs["cw3"], np.float32), 2),
        "wc4": _wlayout1_split(np.asarray(inputs["cw4"], np.float32)),
        "wr1": _wlayout3(np.asarray(inputs["rw1"], np.float32)).astype(np.float16),
        "wr2": _wlayout3(np.asarray(inputs["rw2"], np.float32)).astype(np.float16),
        "wr3": _wlayout3(np.asarray(inputs["rw3"], np.float32)).astype(np.float16),
        "wr4": _wlayout1(np.asarray(inputs["rw4"], np.float32)).astype(np.float16),
        "bc1": _blayout(np.asarray(inputs["cb1"])), "bc2": _blayout(np.asarray(inputs["cb2"])),
        "bc3": _blayout(np.asarray(inputs["cb3"])), "bc4": _blayout(np.asarray(inputs["cb4"])),
        "br1": _blayout(np.asarray(inputs["rb1"])), "br2": _blayout(np.asarray(inputs["rb2"])),
        "br3": _blayout(np.asarray(inputs["rb3"])), "br4": _blayout(np.asarray(inputs["rb4"])),
    }
    x2s = _shard_features(f2, R2)
    x1s = _shard_features(f1, R1)
    in1 = []
    for i in range(8):
        m = dict(wglob)
        x2hi = x2s[i].astype(np.float16)
        m["x2ph"] = x2hi
        m["x2pl"] = (x2s[i] - x2hi.astype(np.float32)).astype(np.float16)
        m["x1p"] = x1s[i].astype(np.float16)
        in1.append(m)

    res1 = run_bass_kernel_spmd(nc1, in1, core_ids=list(range(8)))
    last_exec_ns["phase1"] = res1.exec_time_ns
    classification = np.concatenate(
        [res1.results[i]["class_out"] for i in range(8)], 0)      # (128,128)
    field = np.concatenate(
        [res1.results[i]["field_out"] for i in range(8)], 1)      # (2,256,256)

    uxT = _upsample_weights()
    iota = -(np.arange(128, dtype=np.float32)[:, None]
             + np.array([0.0, 128.0], np.float32)[None, :])
    classT = np.ascontiguousarray(classification.T)
    fieldT = np.ascontiguousarray(np.transpose(field, (0, 2, 1)).astype(np.float16))
    in2 = []
    for i in range(8):
        pts = cont[75 * i:75 * (i + 1)].reshape(NREAL, 2)
        c0 = np.zeros((2, NPTS), np.float32)
        c0[0, :NREAL] = pts[:, 0]
        c0[1, :NREAL] = pts[:, 1]
        in2.append({
            "classT": classT, "fieldT": fieldT, "cont0": c0, "uxT": uxT,
            "uyT": np.ascontiguousarray(uxT[:, 32 * i:32 * (i + 1)]),
            "iota_neg": np.ascontiguousarray(iota.astype(np.float32)),
        })

    res2 = run_bass_kernel_spmd(nc2, in2, core_ids=list(range(8)))
    last_exec_ns["phase2"] = res2.exec_time_ns
    binary_map = np.concatenate(
        [res2.results[i]["brows"] for i in range(8)], 0)          # (256,256)
    conts = []
    for i in range(8):
        co = res2.results[i]["cont_out"]
        conts.append(np.stack([co[0, :NREAL], co[1, :NREAL]], -1).reshape(75, 64, 2))
    contours = np.concatenate(conts, 0)                           # (600,64,2)

    return (classification.reshape(1, 1, 128, 128).astype(np.float32),
            binary_map.reshape(1, 1, 256, 256).astype(np.float32),
            contours.reshape(1, 600, 64, 2).astype(np.float32))
